# revision 38
# baseline (speedup 1.0000x reference)
"""GQA (grouped-query attention) Trainium2 kernel, 8-core SPMD.

Problem: B=4, T=2048, d_model=2048, 32 Q heads, 8 KV heads, d_k=64, causal.
Sharding: core = (batch b, half-of-KV-heads h): 8 cores = 4 batches x 2 halves.
Each core computes its 4 KV heads (16 Q heads) for its batch and the partial
output o_half @ Wo_half (row-parallel Wo); host sums the two halves per batch
and adds bo.

Device-side design (per core):
  - x^T (pre-transposed + bf16-cast on host) resident in SBUF.
  - k^T = Wk^T x^T and v = x Wv projections first (v gets a ones column
    appended so the PV matmul also produces the softmax denominator).
  - Per 512-wide query tile j: per head, scores are computed transposed
    (s^T[tk, tq], K=d_k contraction), exp on the scalar engine (no max
    subtraction: |scores| <= ~2 here), causal masking by multiplying the
    diagonal-crossing tiles with a sliding window of a precomputed 0/1
    mask, PV matmul against ones-augmented v -> o^T rows + sums row.
  - Softmax division: per-tile-j the 16 heads' sums rows are gathered
    into one [16, TQ] tile, one batched DVE reciprocal, bf16 cast, then
    per head a GpSimd partition_broadcast and an in-place DVE multiply.
  - Software pipelining: the O-projection matmul groups of tile j-1 and
    the Q-projection groups of tile j+1 are interleaved between heads of
    tile j, so the TensorEngine stays busy (and HAM-warm) while ACT
    computes exp.
  - Causality skips fully-masked (tk > all tq) score/PV tiles entirely.
"""

import numpy as np
import ml_dtypes
from contextlib import ExitStack

B, T, D = 4, 2048, 2048
NKV, NREP, DK = 8, 4, 64
HALF_KV = 4                  # kv heads per core
NQH = HALF_KV * NREP         # 16 q heads per core
QD = NQH * DK                # 1024 q dims per core
KVD = HALF_KV * DK           # 256 kv dims per core
NCORES = 8
CD = D // 128                # 16 contraction chunks over d_model
CT = T // 128                # 16 token chunks of 128
TQ = 512                     # query tile width
NTQ = T // TQ                # 4 query tiles
SCALE = 1.0 / np.sqrt(DK)

BF16 = ml_dtypes.bfloat16

_cache = {}


def _body(ctx, tc, aps):
    import concourse.mybir as mybir
    from concourse.bass import ts, ds

    nc = tc.nc
    f32 = mybir.dt.float32
    bf16 = mybir.dt.bfloat16
    xT, Wq, bqv, Wk, bkv, Wv, bv, Wo, out = (
        aps["xT"], aps["Wq"], aps["bq"], aps["Wk"], aps["bk"], aps["Wv"],
        aps["bv"], aps["Wo"], aps["out"])

    # ---- pools ----------------------------------------------------------
    rp = ctx.enter_context(tc.tile_pool(name="res", bufs=1))
    qp = ctx.enter_context(tc.tile_pool(name="qt", bufs=2))
    op = ctx.enter_context(tc.tile_pool(name="ot", bufs=2))
    ptp = ctx.enter_context(tc.tile_pool(name="pt", bufs=6))
    sp = ctx.enter_context(tc.tile_pool(name="sm", bufs=2))
    dvp = ctx.enter_context(tc.tile_pool(name="dv", bufs=2))
    wp = ctx.enter_context(tc.tile_pool(name="wk", bufs=2))
    pp = ctx.enter_context(tc.tile_pool(name="ps", bufs=5, space="PSUM"))
    po = ctx.enter_context(tc.tile_pool(name="po", bufs=3, space="PSUM"))

    # ---- resident tiles -------------------------------------------------
    xT_sb = rp.tile([128, CD, T], bf16, tag="xT")           # 64 KiB/part
    Wq_sb = rp.tile([128, CD, QD], bf16, tag="Wq")          # 32 KiB/part
    Wo_sb = rp.tile([128, QD // 128, D], bf16, tag="Wo")    # 32 KiB/part
    kT_sb = rp.tile([128, KVD // 128, T], bf16, tag="kT")
    v_sb = rp.tile([128, CT, HALF_KV, DK + 1], bf16, tag="v")
    bq_sb = rp.tile([128, QD // 128], f32, tag="bq")
    bk_sb = rp.tile([128, KVD // 128], f32, tag="bk")
    bv_sb = rp.tile([1, KVD], bf16, tag="bv")
    ones_b = rp.tile([1, 128], bf16, tag="ones_b")
    # one wide causal window: wmask[p, g] = (g - 384 >= p); the mask for
    # diagonal tile i (tk0 = tq0 + 128*i) is wmask[:, 384-128*i : 896-128*i]
    wmask = rp.tile([128, TQ + 384], bf16, tag="masks")
    # Wk/Wv share the qT tag: their slots are recycled into qT buffers
    # once the K/V projections are done.
    Wk_sb = qp.tile([128, CD, KVD], bf16, tag="qT")
    Wv_sb = qp.tile([128, CD, KVD], bf16, tag="qT")

    for c in range(CD):
        nc.sync.dma_start(xT_sb[:, c, :], xT[c * 128:(c + 1) * 128, :])
        nc.sync.dma_start(Wq_sb[:, c, :], Wq[c * 128:(c + 1) * 128, :])
        nc.sync.dma_start(Wk_sb[:, c, :], Wk[c * 128:(c + 1) * 128, :])
        nc.sync.dma_start(Wv_sb[:, c, :], Wv[c * 128:(c + 1) * 128, :])
    for c in range(QD // 128):
        nc.sync.dma_start(Wo_sb[:, c, :], Wo[c * 128:(c + 1) * 128, :])
        nc.sync.dma_start(bq_sb[:, c:c + 1], bqv[c, :].unsqueeze(-1))
    for c in range(KVD // 128):
        nc.sync.dma_start(bk_sb[:, c:c + 1], bkv[c, :].unsqueeze(-1))
    nc.sync.dma_start(bv_sb[:, :], bv[:, :])

    nc.vector.memset(ones_b[:, :], 1.0)
    nc.vector.memset(v_sb[:, :, :, DK:DK + 1], 1.0)
    nc.vector.memset(wmask[:, :], 1.0)
    nc.gpsimd.affine_select(
        out=wmask[:, :], in_=wmask[:, :],
        compare_op=mybir.AluOpType.is_ge, fill=0.0,
        base=-384, pattern=[[1, TQ + 384]], channel_multiplier=-1)

    # ---- K^T projection -------------------------------------------------
    for m in range(KVD // 128):
        for n in range(T // TQ):
            ps = pp.tile([128, TQ], f32, tag="ps")
            for c in range(CD):
                nc.tensor.matmul(ps[:, :],
                                 Wk_sb[:, c, ts(m, 128)],
                                 xT_sb[:, c, ts(n, TQ)],
                                 start=(c == 0), stop=(c == CD - 1))
            nc.vector.tensor_scalar_add(kT_sb[:, m, ts(n, TQ)], ps[:, :],
                                        bk_sb[:, m:m + 1])

    # ---- V projection (normal layout) -----------------------------------
    for mt in range(CT):
        ps = pp.tile([128, KVD], f32, tag="ps")
        for c in range(CD):
            nc.tensor.matmul(ps[:, :],
                             xT_sb[:, c, ts(mt, 128)],
                             Wv_sb[:, c, :],
                             start=(c == 0), stop=False)
        nc.tensor.matmul(ps[:, :], ones_b[:, :], bv_sb[:, :],
                         start=False, stop=True)
        nc.vector.tensor_copy(v_sb[:, mt, :, 0:DK],
                              ps[:, :].rearrange("p (h d) -> p h d",
                                                 h=HALF_KV))

    # ---- pipelined per-query-tile main loop -----------------------------
    def qproj_group(jj, qT_tile, m):
        # generator: one PE matmul per next() so it can be paced as filler
        ps = pp.tile([128, TQ], f32, tag="ps", name=f"q{jj}_{m}")
        for c in range(CD):
            nc.tensor.matmul(ps[:, :],
                             Wq_sb[:, c, ts(m, 128)],
                             xT_sb[:, c, ds(jj * TQ, TQ)],
                             start=(c == 0), stop=(c == CD - 1))
            if c < CD - 1:
                yield
        nc.vector.tensor_scalar_add(qT_tile[:, m, :], ps[:, :],
                                    bq_sb[:, m:m + 1])
        yield

    def oproj_group(jj, oT_tile, mt, n):
        ps = pp.tile([128, TQ], f32, tag="ps", name=f"o{jj}_{mt}_{n}")
        for c in range(QD // 128):
            nc.tensor.matmul(ps[:, :],
                             oT_tile[:, c, ts(mt, 128)],
                             Wo_sb[:, c, ts(n, TQ)],
                             start=(c == 0), stop=(c == QD // 128 - 1))
            if c < QD // 128 - 1:
                yield
        os_ = wp.tile([128, TQ], f32, tag="os", name=f"os{jj}_{mt}_{n}")
        nc.vector.tensor_copy(os_[:, :], ps[:, :])
        nc.sync.dma_start(
            out[ds(jj * TQ + mt * 128, 128), ts(n, TQ)], os_[:, :])
        yield

    def filler_stream(j, qT_tiles, oT_tiles):
        # one yield per PE matmul: O-proj of tile j-1, then q-proj of j+1
        if j > 0:
            for mt in range(TQ // 128):
                for n in range(D // TQ):
                    yield from oproj_group(j - 1, oT_tiles[j - 1], mt, n)
        if j < NTQ - 1:
            for m in range(QD // 128):
                yield from qproj_group(j + 1, qT_tiles[j + 1], m)

    qT_tiles = {}
    oT_tiles = {}
    # prologue: q^T for tile 0 (drain the generators back-to-back)
    qT_tiles[0] = qp.tile([128, QD // 128, TQ], bf16, tag="qT", name="qT_t0")
    for m in range(QD // 128):
        for _ in qproj_group(0, qT_tiles[0], m):
            pass

    for j in range(NTQ):
        qT_sb = qT_tiles[j]
        oT_sb = op.tile([128, QD // 128, TQ], bf16, tag="oT")
        oT_tiles[j] = oT_sb
        if j < NTQ - 1:
            qT_tiles[j + 1] = qp.tile([128, QD // 128, TQ], bf16, tag="qT",
                                      name=f"qT_t{j+1}")
        nkeep = 4 * j + 4
        filler = filler_stream(j, qT_tiles, oT_tiles)
        n_fill = (128 if j > 0 else 0) + (128 if j < NTQ - 1 else 0) + 24
        n_cks = NQH * nkeep
        fill_acc = 0.0
        fill_rate = n_fill / n_cks

        def fill(k):
            for _ in range(k):
                if next(filler, "done") == "done":
                    break

        for hq in range(NQH):
            kv = hq // NREP
            kb = (kv % 2) * 64
            kTsl = kT_sb[kb:kb + 64, kv // 2, :]
            qsl = qT_sb[(hq % 2) * 64:(hq % 2) * 64 + 64, hq // 2, :]
            if (hq % 2) != (kv % 2):
                # matmul needs lhsT/rhs on the same base partition
                qst = sp.tile([128, TQ], bf16, tag="st")
                nc.vector.tensor_copy(qst[kb:kb + 64, :], qsl)
                qsl = qst[kb:kb + 64, :]
            o65 = po.tile([65, TQ], f32, tag="o65")
            pTs = {}
            for ck in range(nkeep):
                ss = pp.tile([128, TQ], f32, tag="ps")
                nc.tensor.matmul(ss[:, :],
                                 kTsl[:, ts(ck, 128)], qsl[:, :],
                                 start=True, stop=True)
                pT = ptp.tile([128, TQ], bf16, tag="pT")
                nc.scalar.activation(pT[:, :], ss[:, :],
                                     mybir.ActivationFunctionType.Exp,
                                     scale=SCALE)
                di = ck - 4 * j
                if di >= 0:
                    nc.vector.tensor_mul(pT[:, :], pT[:, :],
                                         wmask[:, ds(384 - 128 * di, TQ)])
                pTs[ck] = pT
                # skewed PV: consume the previous chunk's probabilities so
                # the PE never waits on this chunk's exp
                if ck > 0:
                    nc.tensor.matmul(o65[:, :],
                                     v_sb[:, ck - 1, kv, :], pTs[ck - 1][:, :],
                                     start=(ck - 1 == 0), stop=False)
                    del pTs[ck - 1]
                fill_acc += fill_rate
                k = int(fill_acc)
                fill_acc -= k
                fill(k)
            nc.tensor.matmul(o65[:, :],
                             v_sb[:, nkeep - 1, kv, :], pTs[nkeep - 1][:, :],
                             start=(nkeep == 1), stop=True)
            del pTs[nkeep - 1]
            # softmax division: 1/sums (row 64) broadcast over the 64
            # o^T rows, fused with the psum->sbuf eviction
            srow = dvp.tile([1, TQ], f32, tag="sr")
            nc.vector.tensor_copy(srow[:, :], o65[64:65, :])
            rrow = dvp.tile([1, TQ], f32, tag="rr")
            nc.vector.reciprocal_approx_fast(rrow[:, :], srow[:, :])
            bcs = dvp.tile([64, TQ], f32, tag="bc")
            nc.gpsimd.partition_broadcast(bcs[:, :], rrow[:, :])
            nc.vector.tensor_mul(
                oT_sb[(hq % 2) * 64:(hq % 2) * 64 + 64, hq // 2, :],
                o65[0:64, :], bcs[:, :])
        fill(n_fill)

    # epilogue: O-projection of the last tile
    for mt in range(TQ // 128):
        for n in range(D // TQ):
            for _ in oproj_group(NTQ - 1, oT_tiles[NTQ - 1], mt, n):
                pass


def _build():
    import concourse.mybir as mybir
    import concourse.tile as tile
    from concourse import bacc

    nc = bacc.Bacc("TRN2", target_bir_lowering=False, debug=False,
                   num_devices=NCORES)
    f32, bf16 = mybir.dt.float32, mybir.dt.bfloat16
    aps = {
        "xT": nc.dram_tensor("xT", (D, T), bf16, kind="ExternalInput").ap(),
        "Wq": nc.dram_tensor("Wq", (D, QD), bf16, kind="ExternalInput").ap(),
        "bq": nc.dram_tensor("bq", (QD // 128, 128), f32,
                             kind="ExternalInput").ap(),
        "Wk": nc.dram_tensor("Wk", (D, KVD), bf16, kind="ExternalInput").ap(),
        "bk": nc.dram_tensor("bk", (KVD // 128, 128), f32,
                             kind="ExternalInput").ap(),
        "Wv": nc.dram_tensor("Wv", (D, KVD), bf16, kind="ExternalInput").ap(),
        "bv": nc.dram_tensor("bv", (1, KVD), bf16, kind="ExternalInput").ap(),
        "Wo": nc.dram_tensor("Wo", (QD, D), bf16, kind="ExternalInput").ap(),
        "out": nc.dram_tensor("out", (T, D), f32, kind="ExternalOutput").ap(),
    }
    with tile.TileContext(nc) as tc:
        with ExitStack() as ctx:
            _body(ctx, tc, aps)
    nc.compile()
    return nc


def _get_nc():
    if "nc" not in _cache:
        _cache["nc"] = _build()
    return _cache["nc"]


def kernel(x, Wq, bq, Wk, bk, Wv, bv, Wo, bo, **_):
    from concourse.bass_utils import run_bass_kernel_spmd

    x = np.asarray(x, np.float32)
    in_maps = []
    for core in range(NCORES):
        b, h = core // 2, core % 2
        in_maps.append({
            "xT": np.ascontiguousarray(np.asarray(x[b]).T).astype(BF16),
            "Wq": np.asarray(Wq[:, h * QD:(h + 1) * QD], np.float32).astype(BF16),
            "bq": np.asarray(bq[h * QD:(h + 1) * QD], np.float32).reshape(
                QD // 128, 128),
            "Wk": np.asarray(Wk[:, h * KVD:(h + 1) * KVD], np.float32).astype(BF16),
            "bk": np.asarray(bk[h * KVD:(h + 1) * KVD], np.float32).reshape(
                KVD // 128, 128),
            "Wv": np.asarray(Wv[:, h * KVD:(h + 1) * KVD], np.float32).astype(BF16),
            "bv": np.asarray(bv[h * KVD:(h + 1) * KVD], np.float32).reshape(
                1, KVD).astype(BF16),
            "Wo": np.asarray(Wo[h * QD:(h + 1) * QD, :], np.float32).astype(BF16),
        })
    nc = _get_nc()
    res = run_bass_kernel_spmd(nc, in_maps, core_ids=list(range(NCORES)))
    bo = np.asarray(bo, np.float32)
    outs = [np.asarray(res.results[c]["out"], np.float32)
            for c in range(NCORES)]
    return np.stack([outs[2 * b] + outs[2 * b + 1] + bo
                     for b in range(B)], axis=0)
